# revision 22
# baseline (speedup 1.0000x reference)
"""Bidirectional Mamba layer on 8 Trainium2 NeuronCores.

Sharding: data-parallel over batch (8 batches -> 8 cores). Each core runs
both directions (fwd on x, bwd on time-reversed x) for its batch.

v3: engine-rebalanced + software-pipelined across directions.
  - depthwise conv on PE (diag-block matmuls, PSUM tap accumulation)
  - y-mul/y-add/w-mul/gate-mul on the Pool engine; PSUM evacuations on
    ACT (Copy is in every ACT table -> no table reloads); scans + b-mul
    + tensor_scalar on DVE
  - softplus as Exp+Ln (same ACT table as the 256 scan exps)
  - y initialized to uc*D in the prelude (drops the gate add, frees ucT)
  - z parked in scratch DRAM (f: pre-silu'd; b: raw, silu at gate)
  - dir-b's GEMM1+conv are emitted interleaved into dir-f's scan loop
    (engines execute in program order, so emission order is the schedule);
    b's conv nonlinearity uses the tanh identity silu(x)=x*(1+tanh(x/2))/2
    so it shares an ACT table with the concurrent scan exps
  - big per-direction arrays (delta/w/y) live in tag-rotated pools
"""

import sys

sys.path.insert(0, "/opt/trn_rl_repo")

import numpy as np
import ml_dtypes

import concourse.bass as bass
import concourse.mybir as mybir
import bass_rust
from concourse import tile
from concourse.bass_utils import run_bass_kernel_spmd

BF16 = mybir.dt.bfloat16
F32 = mybir.dt.float32
AF = mybir.ActivationFunctionType
OP = mybir.AluOpType

D_MODEL = 512
D_INNER = 1024
D_STATE = 16
D_CONV = 4
DT_RANK = 32
BATCH = 8
SEQ = 1024

P = 128
NC_D = D_INNER // P  # 8 d-chunks
NN = SEQ // 512      # 2 psum-free chunks


def _dir_params(nc, d):
    ps = {
        "inwT": nc.declare_dram_parameter(f"inwT_{d}", [D_MODEL, 2 * D_INNER], BF16, isOutput=False),
        "xpb": nc.declare_dram_parameter(f"xpb_{d}", [P, NC_D * 64], BF16, isOutput=False),
        "dtwT": nc.declare_dram_parameter(f"dtwT_{d}", [DT_RANK, D_INNER], BF16, isOutput=False),
        "outwT": nc.declare_dram_parameter(f"outwT_{d}", [D_INNER, D_MODEL], BF16, isOutput=False),
        "smf": nc.declare_dram_parameter(f"smf_{d}", [P, NC_D * 20], F32, isOutput=False),
        "convdiag": nc.declare_dram_parameter(f"convdiag_{d}", [D_CONV * P, D_INNER], BF16, isOutput=False),
        "xT": nc.declare_dram_parameter(f"xT_{d}", [D_MODEL, SEQ], BF16, isOutput=False),
        "out": nc.declare_dram_parameter(f"out_{d}", [SEQ, D_MODEL], F32, isOutput=True),
    }
    ps["zscr"] = nc.dram_tensor(f"zscr_{d}", [D_INNER, SEQ], BF16)
    return ps


def _load_weights(tc, pools, p, d):
    nc = tc.nc
    cst, trans = pools["cst"], pools["trans"]
    st = {}
    st["inwT"] = [trans.tile([P, 2 * D_INNER], BF16, tag="inwT", name=f"inwT{d}{k}", bufs=4) for k in range(4)]
    st["xT"] = [trans.tile([P, SEQ], BF16, tag="xT", name=f"xT{d}{k}", bufs=4) for k in range(4)]
    for k in range(4):
        nc.sync.dma_start(st["inwT"][k][:], p["inwT"][k * P:(k + 1) * P, :])
        nc.sync.dma_start(st["xT"][k][:], p["xT"][k * P:(k + 1) * P, :])
    st["convdiag"] = [trans.tile([P, D_INNER], BF16, tag="cvd", name=f"cvd{d}{k}", bufs=4) for k in range(D_CONV)]
    for k in range(D_CONV):
        nc.sync.dma_start(st["convdiag"][k][:], p["convdiag"][k * P:(k + 1) * P, :])
    smf = cst.tile([P, NC_D * 20], F32, tag=f"smf{d}", name=f"smf{d}")
    nc.sync.dma_start(smf[:], p["smf"][:])
    xpb = cst.tile([P, NC_D * 64], BF16, tag=f"xpb{d}", name=f"xpb{d}")
    nc.sync.dma_start(xpb[:], p["xpb"][:])
    st["smf"] = smf
    st["xpb"] = xpb
    st["dtwT"] = cst.tile([DT_RANK, D_INNER], BF16, tag=f"dtwT{d}", name=f"dtwT{d}")
    nc.sync.dma_start(st["dtwT"][:], p["dtwT"][:])

    st["delta"] = [pools["big"].tile([P, SEQ], BF16, tag="delta", name=f"delta{d}{c}", bufs=12) for c in range(NC_D)]
    st["w"] = [pools["big"].tile([P, SEQ], BF16, tag="w", name=f"w{d}{c}", bufs=10) for c in range(NC_D)]
    st["bc_bf"] = cst.tile([2 * D_STATE, SEQ], BF16, tag=f"bc_bf{d}", name=f"bc_bf{d}")
    st["dt_bf"] = trans.tile([DT_RANK, SEQ], BF16, tag="dt_bf", name=f"dt_bf{d}", bufs=1)
    st["uT"] = [trans.tile([P, SEQ + D_CONV - 1], BF16, tag="uT", name=f"uT{d}{c}", bufs=4) for c in range(NC_D)]
    st["ucT"] = [trans.tile([P, SEQ], BF16, tag="ucT", name=f"ucT{d}{c}", bufs=8) for c in range(NC_D)]
    for c in range(NC_D):
        nc.vector.memset(st["uT"][c][:, 0:D_CONV - 1], 0.0)
    return st


def _gemm1_conv_units(tc, pools, p, d, st, overlap, defer_z=False):
    """Yield after each GEMM1 (m,n) unit and each conv (c,n) unit.

    overlap=False: conv nonlinearity is a direct ACT Silu; z is silu'd at
    staging time. overlap=True (emitted amid the other direction's scan
    exps): conv uses the tanh identity, z is staged raw.
    """
    nc = tc.nc
    psp, sp = pools["psum"], pools["sp"]
    inwT, xT, uT, ucT = st["inwT"], st["xT"], st["uT"], st["ucT"]

    def g1_unit(m, n):
        pt = psp.tile([P, 512], F32, tag="g1", name="g1", bufs=2)
        for k in range(4):
            nc.tensor.matmul(
                pt[:], inwT[k][:, m * P:(m + 1) * P],
                xT[k][:, n * 512:(n + 1) * 512],
                start=(k == 0), stop=(k == 3),
            )
        if m < NC_D:
            ueng = nc.vector if overlap else nc.scalar
            if overlap:
                nc.vector.tensor_copy(
                    uT[m][:, D_CONV - 1 + n * 512: D_CONV - 1 + (n + 1) * 512], pt[:]
                )
            else:
                nc.scalar.copy(
                    uT[m][:, D_CONV - 1 + n * 512: D_CONV - 1 + (n + 1) * 512], pt[:]
                )
        else:
            zst = sp.tile([P, 512], BF16, tag="zst", name="zst", bufs=2)
            if overlap:
                nc.scalar.copy(zst[:], pt[:])
            else:
                nc.scalar.activation(zst[:], pt[:], AF.Silu)
            nc.sync.dma_start(
                p["zscr"][(m - NC_D) * P:(m - NC_D + 1) * P, n * 512:(n + 1) * 512],
                zst[:],
            )

    def cv_unit(c, n):
        pt = psp.tile([P, 512], F32, tag="cv", name="cv", bufs=1)
        for k in range(D_CONV):
            nc.tensor.matmul(
                pt[:], st["convdiag"][k][:, c * P:(c + 1) * P],
                uT[c][:, k + n * 512: k + n * 512 + 512],
                start=(k == 0), stop=(k == D_CONV - 1),
            )
        sl = slice(n * 512, (n + 1) * 512)
        if not overlap:
            nc.scalar.activation(ucT[c][:, sl], pt[:], AF.Silu, bias=st["smf"][:, c * 20 + 16:c * 20 + 17])
        else:
            # silu(x) = x*(1+tanh(x/2))/2; ch = x/2 (+convb/2 bias), th = tanh(x/2)
            ch = sp.tile([P, 512], BF16, tag="ch", name="ch", bufs=2)
            nc.scalar.activation(ch[:], pt[:], AF.Identity, bias=st["smf"][:, c * 20 + 17:c * 20 + 18], scale=0.5)
            th = sp.tile([P, 512], BF16, tag="th", name="th", bufs=2)
            nc.scalar.activation(th[:], pt[:], AF.Tanh, bias=st["smf"][:, c * 20 + 17:c * 20 + 18], scale=0.5)
            t1 = sp.tile([P, 512], BF16, tag="t1", name="t1", bufs=2)
            nc.gpsimd.tensor_scalar(t1[:], th[:], 1.0, None, op0=OP.add)
            nc.gpsimd.tensor_tensor(ucT[c][:, sl], t1[:], ch[:], op=OP.mult)

    # u-part GEMM1 with conv chasing one chunk behind (keeps uT rotation shallow)
    for m in range(NC_D):
        for n in range(NN):
            g1_unit(m, n)
            yield
        if m >= 1:
            for n in range(NN):
                cv_unit(m - 1, n)
                yield
    for n in range(NN):
        cv_unit(NC_D - 1, n)
        yield
    if not defer_z:
        for m in range(NC_D, 2 * NC_D):
            for n in range(NN):
                g1_unit(m, n)
                yield


def _z_units(tc, pools, p, d, st):
    """GEMM1 z-half; raw z staged to scratch DRAM (ACT Copy is in every
    table, so these can be pumped anywhere). The silu happens at the gate
    via the tanh identity."""
    nc = tc.nc
    psp, sp = pools["psum"], pools["sp"]
    inwT, xT = st["inwT"], st["xT"]
    for m in range(NC_D, 2 * NC_D):
        for n in range(NN):
            pt = psp.tile([P, 512], F32, tag="g1", name="g1", bufs=2)
            for k in range(4):
                nc.tensor.matmul(
                    pt[:], inwT[k][:, m * P:(m + 1) * P],
                    xT[k][:, n * 512:(n + 1) * 512],
                    start=(k == 0), stop=(k == 3),
                )
            zst = sp.tile([P, 512], BF16, tag="zst", name="zst", bufs=2)
            nc.vector.tensor_copy(zst[:], pt[:])
            nc.sync.dma_start(
                p["zscr"][(m - NC_D) * P:(m - NC_D + 1) * P, n * 512:(n + 1) * 512],
                zst[:],
            )
            yield


def _g23_units(tc, pools, p, d, st, m_lo, m_hi):
    """GEMM2 (when m_lo==0) + GEMM3/softplus for m in [m_lo, m_hi)."""
    nc = tc.nc
    psp = pools["psum"]
    ucT, dt_bf, bc_bf = st["ucT"], st["dt_bf"], st["bc_bf"]
    if m_lo == 0:
        for n in range(NN):
            pt = psp.tile([64, 512], F32, tag="g2", name="g2", bufs=1)
            for c in range(NC_D):
                nc.tensor.matmul(
                    pt[:], st["xpb"][:, c * 64:(c + 1) * 64],
                    ucT[c][:, n * 512:(n + 1) * 512],
                    start=(c == 0), stop=(c == NC_D - 1),
                )
            nc.vector.tensor_copy(dt_bf[:, n * 512:(n + 1) * 512], pt[0:DT_RANK, :])
            nc.vector.tensor_copy(bc_bf[:, n * 512:(n + 1) * 512], pt[DT_RANK:64, :])
            yield
    delta = st["delta"]
    for m in range(m_lo, m_hi):
        for n in range(NN):
            pt = psp.tile([P, 512], F32, tag="g3", name="g3", bufs=1)
            nc.tensor.matmul(
                pt[:], st["dtwT"][:, m * P:(m + 1) * P], dt_bf[:, n * 512:(n + 1) * 512],
                start=True, stop=True,
            )
            et = pools["sp"].tile([P, 512], F32, tag="sp_e", name="sp_e", bufs=2)
            nc.scalar.activation(et[:], pt[:], AF.Exp, bias=st["smf"][:, m * 20 + 18:m * 20 + 19])
            nc.scalar.activation(delta[m][:, n * 512:(n + 1) * 512], et[:], AF.Ln, bias=1.0)
        yield


def _wy_init(tc, pools, p, d, st):
    nc = tc.nc
    ucT, delta = st["ucT"], st["delta"]
    st["y"] = [pools["big"].tile([P, SEQ], BF16, tag="y", name=f"y{d}{c}", bufs=10) for c in range(NC_D)]
    for c in range(NC_D):
        nc.vector.tensor_mul(st["w"][c][:], delta[c][:], ucT[c][:])
        nc.vector.tensor_scalar(st["y"][c][:], ucT[c][:], st["smf"][:, c * 20 + 19:c * 20 + 20], None, op0=OP.mult)


def _scan_iter(tc, pools, st, oht, s, cbc_dve=False):
    """One state-index iteration of the selective scan."""
    nc = tc.nc
    psp, bcp, ab = pools["psum"], pools["bc"], pools["ab"]
    delta, w_bf, y_sb, bc_bf = st["delta"], st["w"], st["y"], st["bc_bf"]
    smf = st["smf"]

    Bbc = bcp.tile([P, SEQ], BF16, tag="Bbc", name="Bbc", bufs=2)
    Cbc = bcp.tile([P, SEQ], BF16, tag="Cbc", name="Cbc", bufs=2)
    for src_row, dst in ((s, Bbc), (D_STATE + s, Cbc)):
        for n in range(NN):
            ps = psp.tile([P, 512], F32, tag="bc", name="bcps", bufs=2)
            nc.tensor.matmul(
                ps[:], oht[:, src_row * P:(src_row + 1) * P],
                bc_bf[:, n * 512:(n + 1) * 512],
                start=True, stop=True,
            )
            if cbc_dve and dst is Cbc:
                nc.vector.tensor_copy(dst[:, n * 512:(n + 1) * 512], ps[:])
            else:
                nc.scalar.copy(dst[:, n * 512:(n + 1) * 512], ps[:])
    for c in range(NC_D):
        a_t = ab.tile([P, SEQ], BF16, tag="a", name="a", bufs=2)
        nc.scalar.activation(a_t[:], delta[c][:], AF.Exp, scale=smf[:, c * 20 + s:c * 20 + s + 1])
        b_t = ab.tile([P, SEQ], BF16, tag="b", name="b", bufs=2)
        nc.vector.tensor_mul(b_t[:], w_bf[c][:], Bbc[:])
        h_t = ab.tile([P, SEQ], BF16, tag="h", name="h", bufs=2)
        nc.vector.tensor_tensor_scan(
            h_t[:], a_t[:], b_t[:], 0.0, op0=OP.mult, op1=OP.add
        )
        pr = ab.tile([P, SEQ], BF16, tag="pr", name="pr", bufs=2)
        nc.gpsimd.tensor_tensor(pr[:], h_t[:], Cbc[:], op=OP.mult)
        nc.gpsimd.tensor_tensor(y_sb[c][:], y_sb[c][:], pr[:], op=OP.add)


def _gate_prep(tc, pools, st, p, d):
    """Load outwT and silu'd z (the zin tiles double as the g tiles)."""
    nc = tc.nc
    trans = pools["trans"]
    st["outwT"] = [trans.tile([P, D_MODEL], BF16, tag="outwT", name=f"outwT{d}{c}", bufs=8) for c in range(NC_D)]
    st["zin"] = []
    for c in range(NC_D):
        nc.sync.dma_start(st["outwT"][c][:], p["outwT"][c * P:(c + 1) * P, :])
        zin = trans.tile([P, SEQ], BF16, tag="zin", name=f"zin{d}{c}", bufs=8)
        nc.sync.dma_start(zin[:], p["zscr"][c * P:(c + 1) * P, :])
        st["zin"].append(zin)


def _gate_finish(tc, pools, st, p, d):
    nc = tc.nc
    psp = pools["psum"]
    y_sb, g, outwT = st["y"], st["zin"], st["outwT"]
    for c in range(NC_D):
        # y*silu(z) = y*z*(1+tanh(z/2))*0.5 -- the 0.5 lives in outwT
        th = pools["sp"].tile([P, SEQ], BF16, tag="th2", name="th2", bufs=1)
        nc.scalar.activation(th[:], g[c][:], AF.Tanh, scale=0.5)
        nc.vector.tensor_scalar(th[:], th[:], 1.0, None, op0=OP.add)
        nc.vector.tensor_mul(g[c][:], th[:], g[c][:])
        nc.vector.tensor_mul(g[c][:], y_sb[c][:], g[c][:])
    for m in range(SEQ // P):
        pt = psp.tile([P, D_MODEL], F32, tag="g4", name="g4", bufs=1)
        for c in range(NC_D):
            nc.tensor.matmul(
                pt[:], g[c][:, m * P:(m + 1) * P], outwT[c][:],
                start=(c == 0), stop=(c == NC_D - 1),
            )
        ot = pools["sp"].tile([P, D_MODEL], F32, tag="ot", name="ot", bufs=2)
        nc.vector.tensor_copy(ot[:], pt[:])
        nc.sync.dma_start(p["out"][m * P:(m + 1) * P, :], ot[:])


def _split_excess_waits(nc):
    """walrus accepts at most one sync-wait per instruction (two for
    EventSemaphore); hoist the excess onto injected same-engine NoOps."""
    for f in nc.m.functions:
        for bb in f.blocks:
            new_insts = []
            for inst in bb.instructions:
                si = inst.sync_info
                cap = 2 if isinstance(inst, mybir.InstEventSemaphore) else 1
                if si is not None and len(si.on_wait) > cap:
                    waits = list(si.on_wait)
                    for i, wv in enumerate(waits[:-cap]):
                        nop = mybir.InstNoOp(name=f"{inst.name}-wsplit{i}", ins=[], outs=[])
                        nop.engine = inst.engine
                        nop.sync_info = bass_rust.SyncInfo(on_wait=[wv], on_update=[])
                        new_insts.append(nop)
                    inst.sync_info = bass_rust.SyncInfo(
                        on_wait=waits[-cap:], on_update=list(si.on_update)
                    )
                new_insts.append(inst)
            try:
                bb.instructions = new_insts
            except Exception:
                bb.instructions.clear()
                bb.instructions.extend(new_insts)


def build_bass():
    nc = bass.Bass()
    params = {d: _dir_params(nc, d) for d in ("f", "b")}
    oht_p = nc.declare_dram_parameter("oht", [2 * D_STATE, 2 * D_STATE * P], BF16, isOutput=False)
    with tile.TileContext(nc) as tc:
        with tc.tile_pool(name="cst", bufs=1) as cst, \
             tc.tile_pool(name="trans", bufs=2) as trans, \
             tc.tile_pool(name="big", bufs=10) as big, \
             tc.tile_pool(name="sp", bufs=2) as sp, \
             tc.tile_pool(name="bc", bufs=2) as bc, \
             tc.tile_pool(name="ab", bufs=2) as ab, \
             tc.tile_pool(name="psum", bufs=2, space="PSUM") as psum:
            pools = {"cst": cst, "trans": trans, "big": big, "sp": sp,
                     "bc": bc, "ab": ab, "psum": psum}
            oht = cst.tile([2 * D_STATE, 2 * D_STATE * P], BF16, tag="oht", name="oht")
            nc.sync.dma_start(oht[:], oht_p[:])

            # dir f prelude (u+conv; z-half deferred into the scan pump)
            st_f = _load_weights(tc, pools, params["f"], "f")
            for _ in _gemm1_conv_units(tc, pools, params["f"], "f", st_f,
                                       overlap=False, defer_z=True):
                pass
            for _ in _g23_units(tc, pools, params["f"], "f", st_f, 0, NC_D):
                pass
            _wy_init(tc, pools, params["f"], "f", st_f)

            # dir f scan, with f's z and dir b's GEMM1(u)+conv+z pumped in
            st_b = _load_weights(tc, pools, params["b"], "b")
            import itertools
            gen_b = itertools.chain(
                _z_units(tc, pools, params["f"], "f", st_f),
                _gemm1_conv_units(tc, pools, params["b"], "b", st_b,
                                  overlap=True, defer_z=True),
                _z_units(tc, pools, params["b"], "b", st_b),
                _g23_units(tc, pools, params["b"], "b", st_b, 0, 4),
            )
            for s in range(D_STATE):
                _scan_iter(tc, pools, st_f, oht, s)
                for _ in range(5):
                    if next(gen_b, StopIteration) is StopIteration:
                        break
            for _ in gen_b:
                pass

            _gate_prep(tc, pools, st_f, params["f"], "f")
            _gate_finish(tc, pools, st_f, params["f"], "f")
            for _ in _g23_units(tc, pools, params["b"], "b", st_b, 4, NC_D):
                pass
            _wy_init(tc, pools, params["b"], "b", st_b)
            _gate_prep(tc, pools, st_b, params["b"], "b")
            for s in range(D_STATE):
                _scan_iter(tc, pools, st_b, oht, s)
            _gate_finish(tc, pools, st_b, params["b"], "b")
    _split_excess_waits(nc)
    return nc


def _prep_dir(w):
    bf = ml_dtypes.bfloat16
    in_w, conv_w, conv_b, xp_w, dt_w, dt_b, A_log, Dp, out_w = w
    cw = np.asarray(conv_w, np.float32)
    convdiag = np.zeros((D_CONV, P, NC_D, P), np.float32)
    for k in range(D_CONV):
        for c in range(NC_D):
            convdiag[k, :, c, :] = np.diag(cw[c * P:(c + 1) * P, k])
    A = -np.exp(np.asarray(A_log, np.float64)).astype(np.float64)
    smf = np.zeros((NC_D, P, 20), np.float32)
    for c in range(NC_D):
        sl = slice(c * P, (c + 1) * P)
        smf[c, :, 0:16] = A[sl]
        smf[c, :, 16] = np.asarray(conv_b, np.float32)[sl]
        smf[c, :, 17] = smf[c, :, 16] * 0.5
        smf[c, :, 18] = np.asarray(dt_b, np.float32)[sl]
        smf[c, :, 19] = np.asarray(Dp, np.float32)[sl]
    xpT = np.ascontiguousarray(np.asarray(xp_w, np.float32).T)  # [D_INNER, 64]
    xpb = xpT.reshape(NC_D, P, 64).transpose(1, 0, 2).reshape(P, NC_D * 64)
    return {
        "inwT": np.ascontiguousarray(in_w.T).astype(bf),
        "xpb": np.ascontiguousarray(xpb).astype(bf),
        "dtwT": np.ascontiguousarray(dt_w.T).astype(bf),
        "outwT": (np.ascontiguousarray(out_w.T) * 0.5).astype(bf),
        "convdiag": convdiag.reshape(D_CONV * P, D_INNER).astype(bf),
        "smf": np.ascontiguousarray(smf.transpose(1, 0, 2).reshape(P, NC_D * 20)),
    }


_CACHED = {}


def kernel(
    x,
    in_w_f, conv_w_f, conv_b_f, xp_w_f, dt_w_f, dt_b_f, A_log_f, D_f, out_w_f,
    in_w_b, conv_w_b, conv_b_b, xp_w_b, dt_w_b, dt_b_b, A_log_b, D_b, out_w_b,
):
    bf = ml_dtypes.bfloat16
    x = np.asarray(x, dtype=np.float32)

    if "nc" not in _CACHED:
        _CACHED["nc"] = build_bass()
    nc = _CACHED["nc"]

    wf = _prep_dir((in_w_f, conv_w_f, conv_b_f, xp_w_f, dt_w_f, dt_b_f,
                    A_log_f, D_f, out_w_f))
    wb = _prep_dir((in_w_b, conv_w_b, conv_b_b, xp_w_b, dt_w_b, dt_b_b,
                    A_log_b, D_b, out_w_b))
    oht = np.kron(np.eye(2 * D_STATE, dtype=np.float32), np.ones((1, P), np.float32)).astype(bf)

    in_maps = []
    for b in range(BATCH):
        m = {"oht": oht}
        for d, wd in (("f", wf), ("b", wb)):
            for k, v in wd.items():
                m[f"{k}_{d}"] = v
        m["xT_f"] = np.ascontiguousarray(x[b].T).astype(bf)
        m["xT_b"] = np.ascontiguousarray(x[b][::-1].T).astype(bf)
        in_maps.append(m)

    res = run_bass_kernel_spmd(nc, in_maps, core_ids=list(range(BATCH)))
    out = np.empty((BATCH, SEQ, D_MODEL), np.float32)
    for b in range(BATCH):
        rb = res.results[b]
        out[b] = rb["out_f"] + rb["out_b"][::-1]
    return out


# revision 23
# speedup vs baseline: 1.0471x; 1.0471x over previous
"""Bidirectional Mamba layer on 8 Trainium2 NeuronCores.

Sharding: data-parallel over batch (8 batches -> 8 cores). Each core runs
both directions (fwd on x, bwd on time-reversed x) for its batch.

v3: engine-rebalanced + software-pipelined across directions.
  - depthwise conv on PE (diag-block matmuls, PSUM tap accumulation)
  - y-mul/y-add/w-mul/gate-mul on the Pool engine; PSUM evacuations on
    ACT (Copy is in every ACT table -> no table reloads); scans + b-mul
    + tensor_scalar on DVE
  - softplus as Exp+Ln (same ACT table as the 256 scan exps)
  - y initialized to uc*D in the prelude (drops the gate add, frees ucT)
  - z parked in scratch DRAM (f: pre-silu'd; b: raw, silu at gate)
  - dir-b's GEMM1+conv are emitted interleaved into dir-f's scan loop
    (engines execute in program order, so emission order is the schedule);
    b's conv nonlinearity uses the tanh identity silu(x)=x*(1+tanh(x/2))/2
    so it shares an ACT table with the concurrent scan exps
  - big per-direction arrays (delta/w/y) live in tag-rotated pools
"""

import sys

sys.path.insert(0, "/opt/trn_rl_repo")

import numpy as np
import ml_dtypes

import concourse.bass as bass
import concourse.mybir as mybir
import bass_rust
from concourse import tile
from concourse.bass_utils import run_bass_kernel_spmd

BF16 = mybir.dt.bfloat16
F32 = mybir.dt.float32
AF = mybir.ActivationFunctionType
OP = mybir.AluOpType

D_MODEL = 512
D_INNER = 1024
D_STATE = 16
D_CONV = 4
DT_RANK = 32
BATCH = 8
SEQ = 1024

P = 128
NC_D = D_INNER // P  # 8 d-chunks
NN = SEQ // 512      # 2 psum-free chunks


def _dir_params(nc, d):
    ps = {
        "inwT": nc.declare_dram_parameter(f"inwT_{d}", [D_MODEL, 2 * D_INNER], BF16, isOutput=False),
        "xpb": nc.declare_dram_parameter(f"xpb_{d}", [P, NC_D * 64], BF16, isOutput=False),
        "dtwT": nc.declare_dram_parameter(f"dtwT_{d}", [DT_RANK, D_INNER], BF16, isOutput=False),
        "outwT": nc.declare_dram_parameter(f"outwT_{d}", [D_INNER, D_MODEL], BF16, isOutput=False),
        "smf": nc.declare_dram_parameter(f"smf_{d}", [P, NC_D * 20], F32, isOutput=False),
        "convdiag": nc.declare_dram_parameter(f"convdiag_{d}", [D_CONV * P, D_INNER], BF16, isOutput=False),
        "xT": nc.declare_dram_parameter(f"xT_{d}", [D_MODEL, SEQ], BF16, isOutput=False),
        "out": nc.declare_dram_parameter(f"out_{d}", [SEQ, D_MODEL], F32, isOutput=True),
    }
    ps["zscr"] = nc.dram_tensor(f"zscr_{d}", [D_INNER, SEQ], BF16)
    ps["bcscr"] = nc.dram_tensor(f"bcscr_{d}", [2 * D_STATE, SEQ], BF16)
    return ps


def _load_weights(tc, pools, p, d):
    nc = tc.nc
    cst, trans = pools["cst"], pools["trans"]
    st = {}
    st["inwT"] = [trans.tile([P, 2 * D_INNER], BF16, tag="inwT", name=f"inwT{d}{k}", bufs=4) for k in range(4)]
    st["xT"] = [trans.tile([P, SEQ], BF16, tag="xT", name=f"xT{d}{k}", bufs=4) for k in range(4)]
    for k in range(4):
        nc.sync.dma_start(st["inwT"][k][:], p["inwT"][k * P:(k + 1) * P, :])
        nc.sync.dma_start(st["xT"][k][:], p["xT"][k * P:(k + 1) * P, :])
    st["convdiag"] = [trans.tile([P, D_INNER], BF16, tag="cvd", name=f"cvd{d}{k}", bufs=4) for k in range(D_CONV)]
    for k in range(D_CONV):
        nc.sync.dma_start(st["convdiag"][k][:], p["convdiag"][k * P:(k + 1) * P, :])
    smf = cst.tile([P, NC_D * 20], F32, tag=f"smf{d}", name=f"smf{d}")
    nc.sync.dma_start(smf[:], p["smf"][:])
    xpb = cst.tile([P, NC_D * 64], BF16, tag=f"xpb{d}", name=f"xpb{d}")
    nc.sync.dma_start(xpb[:], p["xpb"][:])
    st["smf"] = smf
    st["xpb"] = xpb
    st["dtwT"] = cst.tile([DT_RANK, D_INNER], BF16, tag=f"dtwT{d}", name=f"dtwT{d}")
    nc.sync.dma_start(st["dtwT"][:], p["dtwT"][:])

    st["delta"] = [pools["big"].tile([P, SEQ], BF16, tag="delta", name=f"delta{d}{c}", bufs=12) for c in range(NC_D)]
    st["w"] = [pools["big"].tile([P, SEQ], BF16, tag="w", name=f"w{d}{c}", bufs=10) for c in range(NC_D)]
    st["bc_bf"] = cst.tile([2 * D_STATE, SEQ], BF16, tag=f"bc_bf{d}", name=f"bc_bf{d}")
    st["dt_bf"] = trans.tile([DT_RANK, SEQ], BF16, tag="dt_bf", name=f"dt_bf{d}", bufs=1)
    st["uT"] = [trans.tile([P, SEQ + D_CONV - 1], BF16, tag="uT", name=f"uT{d}{c}", bufs=4) for c in range(NC_D)]
    st["ucT"] = [trans.tile([P, SEQ], BF16, tag="ucT", name=f"ucT{d}{c}", bufs=8) for c in range(NC_D)]
    for c in range(NC_D):
        nc.vector.memset(st["uT"][c][:, 0:D_CONV - 1], 0.0)
    return st


def _gemm1_conv_units(tc, pools, p, d, st, overlap, defer_z=False):
    """Yield after each GEMM1 (m,n) unit and each conv (c,n) unit.

    overlap=False: conv nonlinearity is a direct ACT Silu; z is silu'd at
    staging time. overlap=True (emitted amid the other direction's scan
    exps): conv uses the tanh identity, z is staged raw.
    """
    nc = tc.nc
    psp, sp = pools["psum"], pools["sp"]
    inwT, xT, uT, ucT = st["inwT"], st["xT"], st["uT"], st["ucT"]

    def g1_unit(m, n):
        pt = psp.tile([P, 512], F32, tag="g1", name="g1", bufs=2)
        for k in range(4):
            nc.tensor.matmul(
                pt[:], inwT[k][:, m * P:(m + 1) * P],
                xT[k][:, n * 512:(n + 1) * 512],
                start=(k == 0), stop=(k == 3),
            )
        if m < NC_D:
            nc.scalar.copy(
                uT[m][:, D_CONV - 1 + n * 512: D_CONV - 1 + (n + 1) * 512], pt[:]
            )
        else:
            zst = sp.tile([P, 512], BF16, tag="zst", name="zst", bufs=2)
            if overlap:
                nc.scalar.copy(zst[:], pt[:])
            else:
                nc.scalar.activation(zst[:], pt[:], AF.Silu)
            nc.sync.dma_start(
                p["zscr"][(m - NC_D) * P:(m - NC_D + 1) * P, n * 512:(n + 1) * 512],
                zst[:],
            )

    def cv_unit(c, n):
        pt = psp.tile([P, 512], F32, tag="cv", name="cv", bufs=1)
        for k in range(D_CONV):
            nc.tensor.matmul(
                pt[:], st["convdiag"][k][:, c * P:(c + 1) * P],
                uT[c][:, k + n * 512: k + n * 512 + 512],
                start=(k == 0), stop=(k == D_CONV - 1),
            )
        sl = slice(n * 512, (n + 1) * 512)
        if not overlap:
            nc.scalar.activation(ucT[c][:, sl], pt[:], AF.Silu, bias=st["smf"][:, c * 20 + 16:c * 20 + 17])
        else:
            # silu(x) = x*(1+tanh(x/2))/2; ch = x/2 (+convb/2 bias), th = tanh(x/2)
            ch = sp.tile([P, 512], BF16, tag="ch", name="ch", bufs=2)
            nc.scalar.activation(ch[:], pt[:], AF.Identity, bias=st["smf"][:, c * 20 + 17:c * 20 + 18], scale=0.5)
            th = sp.tile([P, 512], BF16, tag="th", name="th", bufs=2)
            nc.scalar.activation(th[:], pt[:], AF.Tanh, bias=st["smf"][:, c * 20 + 17:c * 20 + 18], scale=0.5)
            t1 = sp.tile([P, 512], BF16, tag="t1", name="t1", bufs=2)
            nc.gpsimd.tensor_scalar(t1[:], th[:], 1.0, None, op0=OP.add)
            nc.gpsimd.tensor_tensor(ucT[c][:, sl], t1[:], ch[:], op=OP.mult)

    # u-part GEMM1 with conv chasing one chunk behind (keeps uT rotation shallow)
    for m in range(NC_D):
        for n in range(NN):
            g1_unit(m, n)
            yield
        if m >= 1:
            for n in range(NN):
                cv_unit(m - 1, n)
                yield
    for n in range(NN):
        cv_unit(NC_D - 1, n)
        yield
    if not defer_z:
        for m in range(NC_D, 2 * NC_D):
            for n in range(NN):
                g1_unit(m, n)
                yield


def _z_units(tc, pools, p, d, st):
    """GEMM1 z-half; raw z staged to scratch DRAM (ACT Copy is in every
    table, so these can be pumped anywhere). The silu happens at the gate
    via the tanh identity."""
    nc = tc.nc
    psp, sp = pools["psum"], pools["sp"]
    inwT, xT = st["inwT"], st["xT"]
    for m in range(NC_D, 2 * NC_D):
        for n in range(NN):
            pt = psp.tile([P, 512], F32, tag="g1", name="g1", bufs=2)
            for k in range(4):
                nc.tensor.matmul(
                    pt[:], inwT[k][:, m * P:(m + 1) * P],
                    xT[k][:, n * 512:(n + 1) * 512],
                    start=(k == 0), stop=(k == 3),
                )
            zst = sp.tile([P, 512], BF16, tag="zst", name="zst", bufs=2)
            nc.scalar.copy(zst[:], pt[:])
            nc.sync.dma_start(
                p["zscr"][(m - NC_D) * P:(m - NC_D + 1) * P, n * 512:(n + 1) * 512],
                zst[:],
            )
            yield


def _g23_units(tc, pools, p, d, st, m_lo, m_hi):
    """GEMM2 (when m_lo==0) + GEMM3/softplus for m in [m_lo, m_hi)."""
    nc = tc.nc
    psp = pools["psum"]
    ucT, dt_bf, bc_bf = st["ucT"], st["dt_bf"], st["bc_bf"]
    if m_lo == 0:
        for n in range(NN):
            pt = psp.tile([64, 512], F32, tag="g2", name="g2", bufs=1)
            for c in range(NC_D):
                nc.tensor.matmul(
                    pt[:], st["xpb"][:, c * 64:(c + 1) * 64],
                    ucT[c][:, n * 512:(n + 1) * 512],
                    start=(c == 0), stop=(c == NC_D - 1),
                )
            nc.vector.tensor_copy(dt_bf[:, n * 512:(n + 1) * 512], pt[0:DT_RANK, :])
            nc.vector.tensor_copy(bc_bf[:, n * 512:(n + 1) * 512], pt[DT_RANK:64, :])
            nc.sync.dma_start(p["bcscr"][:, n * 512:(n + 1) * 512],
                              bc_bf[:, n * 512:(n + 1) * 512])
            yield
    delta = st["delta"]
    for m in range(m_lo, m_hi):
        for n in range(NN):
            pt = psp.tile([P, 512], F32, tag="g3", name="g3", bufs=2)
            nc.tensor.matmul(
                pt[:], st["dtwT"][:, m * P:(m + 1) * P], dt_bf[:, n * 512:(n + 1) * 512],
                start=True, stop=True,
            )
            et = pools["sp"].tile([P, 512], F32, tag="sp_e", name="sp_e", bufs=2)
            nc.scalar.activation(et[:], pt[:], AF.Exp, bias=st["smf"][:, m * 20 + 18:m * 20 + 19])
            nc.scalar.activation(delta[m][:, n * 512:(n + 1) * 512], et[:], AF.Ln, bias=1.0)
        yield


def _wy_init(tc, pools, p, d, st):
    nc = tc.nc
    ucT, delta = st["ucT"], st["delta"]
    st["y"] = [pools["big"].tile([P, SEQ], BF16, tag="y", name=f"y{d}{c}", bufs=10) for c in range(NC_D)]
    for c in range(NC_D):
        nc.vector.tensor_mul(st["w"][c][:], delta[c][:], ucT[c][:])
        nc.vector.tensor_scalar(st["y"][c][:], ucT[c][:], st["smf"][:, c * 20 + 19:c * 20 + 20], None, op0=OP.mult)


def _scan_iter(tc, pools, st, p, s):
    """One state-index iteration of the selective scan."""
    nc = tc.nc
    bcp, ab = pools["bc"], pools["ab"]
    delta, w_bf, y_sb = st["delta"], st["w"], st["y"]
    smf = st["smf"]

    Bbc = bcp.tile([P, SEQ], BF16, tag="Bbc", name="Bbc", bufs=2)
    Cbc = bcp.tile([P, SEQ], BF16, tag="Cbc", name="Cbc", bufs=2)
    for src_row, dst in ((s, Bbc), (D_STATE + s, Cbc)):
        nc.sync.dma_start(
            dst[:], p["bcscr"][src_row:src_row + 1, :].broadcast_to((P, SEQ))
        )
    for c in range(NC_D):
        a_t = ab.tile([P, SEQ], BF16, tag="a", name="a", bufs=2)
        nc.scalar.activation(a_t[:], delta[c][:], AF.Exp, scale=smf[:, c * 20 + s:c * 20 + s + 1])
        b_t = ab.tile([P, SEQ], BF16, tag="b", name="b", bufs=2)
        nc.vector.tensor_mul(b_t[:], w_bf[c][:], Bbc[:])
        h_t = ab.tile([P, SEQ], BF16, tag="h", name="h", bufs=2)
        nc.vector.tensor_tensor_scan(
            h_t[:], a_t[:], b_t[:], 0.0, op0=OP.mult, op1=OP.add
        )
        pr = ab.tile([P, SEQ], BF16, tag="pr", name="pr", bufs=2)
        nc.gpsimd.tensor_tensor(pr[:], h_t[:], Cbc[:], op=OP.mult)
        nc.gpsimd.tensor_tensor(y_sb[c][:], y_sb[c][:], pr[:], op=OP.add)


def _gate_prep(tc, pools, st, p, d):
    """Load outwT and silu'd z (the zin tiles double as the g tiles)."""
    nc = tc.nc
    trans = pools["trans"]
    st["outwT"] = [trans.tile([P, D_MODEL], BF16, tag="outwT", name=f"outwT{d}{c}", bufs=8) for c in range(NC_D)]
    st["zin"] = []
    for c in range(NC_D):
        nc.sync.dma_start(st["outwT"][c][:], p["outwT"][c * P:(c + 1) * P, :])
        zin = trans.tile([P, SEQ], BF16, tag="zin", name=f"zin{d}{c}", bufs=8)
        nc.sync.dma_start(zin[:], p["zscr"][c * P:(c + 1) * P, :])
        st["zin"].append(zin)


def _gate_finish(tc, pools, st, p, d):
    nc = tc.nc
    psp = pools["psum"]
    y_sb, g, outwT = st["y"], st["zin"], st["outwT"]
    for c in range(NC_D):
        # y*silu(z) = y*z*(1+tanh(z/2))*0.5 -- the 0.5 lives in outwT
        th = pools["sp"].tile([P, SEQ], BF16, tag="th2", name="th2", bufs=1)
        nc.scalar.activation(th[:], g[c][:], AF.Tanh, scale=0.5)
        nc.vector.tensor_scalar(th[:], th[:], 1.0, None, op0=OP.add)
        nc.vector.tensor_mul(g[c][:], th[:], g[c][:])
        nc.vector.tensor_mul(g[c][:], y_sb[c][:], g[c][:])
    for m in range(SEQ // P):
        pt = psp.tile([P, D_MODEL], F32, tag="g4", name="g4", bufs=1)
        for c in range(NC_D):
            nc.tensor.matmul(
                pt[:], g[c][:, m * P:(m + 1) * P], outwT[c][:],
                start=(c == 0), stop=(c == NC_D - 1),
            )
        ot = pools["sp"].tile([P, D_MODEL], F32, tag="ot", name="ot", bufs=2)
        nc.vector.tensor_copy(ot[:], pt[:])
        nc.sync.dma_start(p["out"][m * P:(m + 1) * P, :], ot[:])


def _split_excess_waits(nc):
    """walrus accepts at most one sync-wait per instruction (two for
    EventSemaphore); hoist the excess onto injected same-engine NoOps."""
    for f in nc.m.functions:
        for bb in f.blocks:
            new_insts = []
            for inst in bb.instructions:
                si = inst.sync_info
                cap = 2 if isinstance(inst, mybir.InstEventSemaphore) else 1
                if si is not None and len(si.on_wait) > cap:
                    waits = list(si.on_wait)
                    for i, wv in enumerate(waits[:-cap]):
                        nop = mybir.InstNoOp(name=f"{inst.name}-wsplit{i}", ins=[], outs=[])
                        nop.engine = inst.engine
                        nop.sync_info = bass_rust.SyncInfo(on_wait=[wv], on_update=[])
                        new_insts.append(nop)
                    inst.sync_info = bass_rust.SyncInfo(
                        on_wait=waits[-cap:], on_update=list(si.on_update)
                    )
                new_insts.append(inst)
            try:
                bb.instructions = new_insts
            except Exception:
                bb.instructions.clear()
                bb.instructions.extend(new_insts)


def build_bass():
    nc = bass.Bass()
    params = {d: _dir_params(nc, d) for d in ("f", "b")}
    with tile.TileContext(nc) as tc:
        with tc.tile_pool(name="cst", bufs=1) as cst, \
             tc.tile_pool(name="trans", bufs=2) as trans, \
             tc.tile_pool(name="big", bufs=10) as big, \
             tc.tile_pool(name="sp", bufs=2) as sp, \
             tc.tile_pool(name="bc", bufs=2) as bc, \
             tc.tile_pool(name="ab", bufs=2) as ab, \
             tc.tile_pool(name="psum", bufs=2, space="PSUM") as psum:
            pools = {"cst": cst, "trans": trans, "big": big, "sp": sp,
                     "bc": bc, "ab": ab, "psum": psum}
            # dir f prelude (u+conv; z-half deferred into the scan pump)
            st_f = _load_weights(tc, pools, params["f"], "f")
            for _ in _gemm1_conv_units(tc, pools, params["f"], "f", st_f,
                                       overlap=False, defer_z=True):
                pass
            for _ in _g23_units(tc, pools, params["f"], "f", st_f, 0, NC_D):
                pass
            _wy_init(tc, pools, params["f"], "f", st_f)

            # dir f scan, with f's z and dir b's GEMM1(u)+conv+z pumped in
            st_b = _load_weights(tc, pools, params["b"], "b")
            import itertools
            gen_b = itertools.chain(
                _z_units(tc, pools, params["f"], "f", st_f),
                _gemm1_conv_units(tc, pools, params["b"], "b", st_b,
                                  overlap=True, defer_z=True),
                _z_units(tc, pools, params["b"], "b", st_b),
                _g23_units(tc, pools, params["b"], "b", st_b, 0, 4),
            )
            for s in range(D_STATE):
                _scan_iter(tc, pools, st_f, params["f"], s)
                for _ in range(5):
                    if next(gen_b, StopIteration) is StopIteration:
                        break
            for _ in gen_b:
                pass

            _gate_prep(tc, pools, st_f, params["f"], "f")
            _gate_finish(tc, pools, st_f, params["f"], "f")
            for _ in _g23_units(tc, pools, params["b"], "b", st_b, 4, NC_D):
                pass
            _wy_init(tc, pools, params["b"], "b", st_b)
            _gate_prep(tc, pools, st_b, params["b"], "b")
            for s in range(D_STATE):
                _scan_iter(tc, pools, st_b, params["b"], s)
            _gate_finish(tc, pools, st_b, params["b"], "b")
    _split_excess_waits(nc)
    return nc


def _prep_dir(w):
    bf = ml_dtypes.bfloat16
    in_w, conv_w, conv_b, xp_w, dt_w, dt_b, A_log, Dp, out_w = w
    cw = np.asarray(conv_w, np.float32)
    convdiag = np.zeros((D_CONV, P, NC_D, P), np.float32)
    for k in range(D_CONV):
        for c in range(NC_D):
            convdiag[k, :, c, :] = np.diag(cw[c * P:(c + 1) * P, k])
    A = -np.exp(np.asarray(A_log, np.float64)).astype(np.float64)
    smf = np.zeros((NC_D, P, 20), np.float32)
    for c in range(NC_D):
        sl = slice(c * P, (c + 1) * P)
        smf[c, :, 0:16] = A[sl]
        smf[c, :, 16] = np.asarray(conv_b, np.float32)[sl]
        smf[c, :, 17] = smf[c, :, 16] * 0.5
        smf[c, :, 18] = np.asarray(dt_b, np.float32)[sl]
        smf[c, :, 19] = np.asarray(Dp, np.float32)[sl]
    xpT = np.ascontiguousarray(np.asarray(xp_w, np.float32).T)  # [D_INNER, 64]
    xpb = xpT.reshape(NC_D, P, 64).transpose(1, 0, 2).reshape(P, NC_D * 64)
    return {
        "inwT": np.ascontiguousarray(in_w.T).astype(bf),
        "xpb": np.ascontiguousarray(xpb).astype(bf),
        "dtwT": np.ascontiguousarray(dt_w.T).astype(bf),
        "outwT": (np.ascontiguousarray(out_w.T) * 0.5).astype(bf),
        "convdiag": convdiag.reshape(D_CONV * P, D_INNER).astype(bf),
        "smf": np.ascontiguousarray(smf.transpose(1, 0, 2).reshape(P, NC_D * 20)),
    }


_CACHED = {}


def kernel(
    x,
    in_w_f, conv_w_f, conv_b_f, xp_w_f, dt_w_f, dt_b_f, A_log_f, D_f, out_w_f,
    in_w_b, conv_w_b, conv_b_b, xp_w_b, dt_w_b, dt_b_b, A_log_b, D_b, out_w_b,
):
    bf = ml_dtypes.bfloat16
    x = np.asarray(x, dtype=np.float32)

    if "nc" not in _CACHED:
        _CACHED["nc"] = build_bass()
    nc = _CACHED["nc"]

    wf = _prep_dir((in_w_f, conv_w_f, conv_b_f, xp_w_f, dt_w_f, dt_b_f,
                    A_log_f, D_f, out_w_f))
    wb = _prep_dir((in_w_b, conv_w_b, conv_b_b, xp_w_b, dt_w_b, dt_b_b,
                    A_log_b, D_b, out_w_b))

    in_maps = []
    for b in range(BATCH):
        m = {}
        for d, wd in (("f", wf), ("b", wb)):
            for k, v in wd.items():
                m[f"{k}_{d}"] = v
        m["xT_f"] = np.ascontiguousarray(x[b].T).astype(bf)
        m["xT_b"] = np.ascontiguousarray(x[b][::-1].T).astype(bf)
        in_maps.append(m)

    res = run_bass_kernel_spmd(nc, in_maps, core_ids=list(range(BATCH)))
    out = np.empty((BATCH, SEQ, D_MODEL), np.float32)
    for b in range(BATCH):
        rb = res.results[b]
        out[b] = rb["out_f"] + rb["out_b"][::-1]
    return out


# revision 24
# speedup vs baseline: 1.0583x; 1.0107x over previous
"""Bidirectional Mamba layer on 8 Trainium2 NeuronCores.

Sharding: data-parallel over batch (8 batches -> 8 cores). Each core runs
both directions (fwd on x, bwd on time-reversed x) for its batch.

v3: engine-rebalanced + software-pipelined across directions.
  - depthwise conv on PE (diag-block matmuls, PSUM tap accumulation)
  - y-mul/y-add/w-mul/gate-mul on the Pool engine; PSUM evacuations on
    ACT (Copy is in every ACT table -> no table reloads); scans + b-mul
    + tensor_scalar on DVE
  - softplus as Exp+Ln (same ACT table as the 256 scan exps)
  - y initialized to uc*D in the prelude (drops the gate add, frees ucT)
  - z parked in scratch DRAM (f: pre-silu'd; b: raw, silu at gate)
  - dir-b's GEMM1+conv are emitted interleaved into dir-f's scan loop
    (engines execute in program order, so emission order is the schedule);
    b's conv nonlinearity uses the tanh identity silu(x)=x*(1+tanh(x/2))/2
    so it shares an ACT table with the concurrent scan exps
  - big per-direction arrays (delta/w/y) live in tag-rotated pools
"""

import sys

sys.path.insert(0, "/opt/trn_rl_repo")

import numpy as np
import ml_dtypes

import concourse.bass as bass
import concourse.mybir as mybir
import bass_rust
from concourse import tile
from concourse.bass_utils import run_bass_kernel_spmd

BF16 = mybir.dt.bfloat16
F32 = mybir.dt.float32
AF = mybir.ActivationFunctionType
OP = mybir.AluOpType

D_MODEL = 512
D_INNER = 1024
D_STATE = 16
D_CONV = 4
DT_RANK = 32
BATCH = 8
SEQ = 1024

P = 128
NC_D = D_INNER // P  # 8 d-chunks
NN = SEQ // 512      # 2 psum-free chunks


def _dir_params(nc, d):
    ps = {
        "inwT": nc.declare_dram_parameter(f"inwT_{d}", [D_MODEL, 2 * D_INNER], BF16, isOutput=False),
        "xpb": nc.declare_dram_parameter(f"xpb_{d}", [P, NC_D * 64], BF16, isOutput=False),
        "dtwT": nc.declare_dram_parameter(f"dtwT_{d}", [DT_RANK, D_INNER], BF16, isOutput=False),
        "outwT": nc.declare_dram_parameter(f"outwT_{d}", [D_INNER, D_MODEL], BF16, isOutput=False),
        "smf": nc.declare_dram_parameter(f"smf_{d}", [P, NC_D * 20], F32, isOutput=False),
        "convdiag": nc.declare_dram_parameter(f"convdiag_{d}", [D_CONV * P, D_INNER], BF16, isOutput=False),
        "xT": nc.declare_dram_parameter(f"xT_{d}", [D_MODEL, SEQ], BF16, isOutput=False),
        "out": nc.declare_dram_parameter(f"out_{d}", [SEQ, D_MODEL], F32, isOutput=True),
    }
    ps["zscr"] = nc.dram_tensor(f"zscr_{d}", [D_INNER, SEQ], BF16)
    ps["bcscr"] = nc.dram_tensor(f"bcscr_{d}", [2 * D_STATE, SEQ], BF16)
    return ps


def _load_weights(tc, pools, p, d):
    nc = tc.nc
    cst, trans = pools["cst"], pools["trans"]
    st = {}
    st["inwT"] = [trans.tile([P, 2 * D_INNER], BF16, tag="inwT", name=f"inwT{d}{k}", bufs=4) for k in range(4)]
    st["xT"] = [trans.tile([P, SEQ], BF16, tag="xT", name=f"xT{d}{k}", bufs=4) for k in range(4)]
    for k in range(4):
        nc.sync.dma_start(st["inwT"][k][:], p["inwT"][k * P:(k + 1) * P, :])
        nc.sync.dma_start(st["xT"][k][:], p["xT"][k * P:(k + 1) * P, :])
    st["convdiag"] = [trans.tile([P, D_INNER], BF16, tag="cvd", name=f"cvd{d}{k}", bufs=4) for k in range(D_CONV)]
    for k in range(D_CONV):
        nc.sync.dma_start(st["convdiag"][k][:], p["convdiag"][k * P:(k + 1) * P, :])
    smf = cst.tile([P, NC_D * 20], F32, tag=f"smf{d}", name=f"smf{d}")
    nc.sync.dma_start(smf[:], p["smf"][:])
    xpb = cst.tile([P, NC_D * 64], BF16, tag=f"xpb{d}", name=f"xpb{d}")
    nc.sync.dma_start(xpb[:], p["xpb"][:])
    st["smf"] = smf
    st["xpb"] = xpb
    st["dtwT"] = cst.tile([DT_RANK, D_INNER], BF16, tag=f"dtwT{d}", name=f"dtwT{d}")
    nc.sync.dma_start(st["dtwT"][:], p["dtwT"][:])

    st["delta"] = [pools["big"].tile([P, SEQ], BF16, tag="delta", name=f"delta{d}{c}", bufs=12) for c in range(NC_D)]
    st["w"] = [pools["big"].tile([P, SEQ], BF16, tag="w", name=f"w{d}{c}", bufs=10) for c in range(NC_D)]
    st["bc_bf"] = cst.tile([2 * D_STATE, SEQ], BF16, tag=f"bc_bf{d}", name=f"bc_bf{d}")
    st["dt_bf"] = trans.tile([DT_RANK, SEQ], BF16, tag="dt_bf", name=f"dt_bf{d}", bufs=1)
    st["uT"] = [trans.tile([P, SEQ + D_CONV - 1], BF16, tag="uT", name=f"uT{d}{c}", bufs=4) for c in range(NC_D)]
    st["ucT"] = [trans.tile([P, SEQ], BF16, tag="ucT", name=f"ucT{d}{c}", bufs=8) for c in range(NC_D)]
    for c in range(NC_D):
        nc.vector.memset(st["uT"][c][:, 0:D_CONV - 1], 0.0)
    return st


def _gemm1_conv_units(tc, pools, p, d, st, overlap, defer_z=False):
    """Yield after each GEMM1 (m,n) unit and each conv (c,n) unit.

    overlap=False: conv nonlinearity is a direct ACT Silu; z is silu'd at
    staging time. overlap=True (emitted amid the other direction's scan
    exps): conv uses the tanh identity, z is staged raw.
    """
    nc = tc.nc
    psp, sp = pools["psum"], pools["sp"]
    inwT, xT, uT, ucT = st["inwT"], st["xT"], st["uT"], st["ucT"]

    def g1_unit(m, n):
        pt = psp.tile([P, 512], F32, tag="g1", name="g1", bufs=2)
        for k in range(4):
            nc.tensor.matmul(
                pt[:], inwT[k][:, m * P:(m + 1) * P],
                xT[k][:, n * 512:(n + 1) * 512],
                start=(k == 0), stop=(k == 3),
            )
        if m < NC_D:
            nc.scalar.copy(
                uT[m][:, D_CONV - 1 + n * 512: D_CONV - 1 + (n + 1) * 512], pt[:]
            )
        else:
            zst = sp.tile([P, 512], BF16, tag="zst", name="zst", bufs=2)
            if overlap:
                nc.scalar.copy(zst[:], pt[:])
            else:
                nc.scalar.activation(zst[:], pt[:], AF.Silu)
            nc.sync.dma_start(
                p["zscr"][(m - NC_D) * P:(m - NC_D + 1) * P, n * 512:(n + 1) * 512],
                zst[:],
            )

    def cv_unit(c, n):
        pt = psp.tile([P, 512], F32, tag="cv", name="cv", bufs=1)
        for k in range(D_CONV):
            nc.tensor.matmul(
                pt[:], st["convdiag"][k][:, c * P:(c + 1) * P],
                uT[c][:, k + n * 512: k + n * 512 + 512],
                start=(k == 0), stop=(k == D_CONV - 1),
            )
        sl = slice(n * 512, (n + 1) * 512)
        if not overlap:
            nc.scalar.activation(ucT[c][:, sl], pt[:], AF.Silu, bias=st["smf"][:, c * 20 + 16:c * 20 + 17])
        else:
            # silu(x) = x*(1+tanh(x/2))/2; ch = x/2 (+convb/2 bias), th = tanh(x/2)
            ch = sp.tile([P, 512], BF16, tag="ch", name="ch", bufs=2)
            nc.scalar.activation(ch[:], pt[:], AF.Identity, bias=st["smf"][:, c * 20 + 17:c * 20 + 18], scale=0.5)
            th = sp.tile([P, 512], BF16, tag="th", name="th", bufs=2)
            nc.scalar.activation(th[:], pt[:], AF.Tanh, bias=st["smf"][:, c * 20 + 17:c * 20 + 18], scale=0.5)
            t1 = sp.tile([P, 512], BF16, tag="t1", name="t1", bufs=2)
            nc.gpsimd.tensor_scalar(t1[:], th[:], 1.0, None, op0=OP.add)
            nc.gpsimd.tensor_tensor(ucT[c][:, sl], t1[:], ch[:], op=OP.mult)

    # u-part GEMM1 with conv chasing one chunk behind (keeps uT rotation shallow)
    for m in range(NC_D):
        for n in range(NN):
            g1_unit(m, n)
            yield
        if m >= 1:
            for n in range(NN):
                cv_unit(m - 1, n)
                yield
    for n in range(NN):
        cv_unit(NC_D - 1, n)
        yield
    if not defer_z:
        for m in range(NC_D, 2 * NC_D):
            for n in range(NN):
                g1_unit(m, n)
                yield


def _z_units(tc, pools, p, d, st):
    """GEMM1 z-half; raw z staged to scratch DRAM (ACT Copy is in every
    table, so these can be pumped anywhere). The silu happens at the gate
    via the tanh identity."""
    nc = tc.nc
    psp, sp = pools["psum"], pools["sp"]
    inwT, xT = st["inwT"], st["xT"]
    for m in range(NC_D, 2 * NC_D):
        for n in range(NN):
            pt = psp.tile([P, 512], F32, tag="g1", name="g1", bufs=2)
            for k in range(4):
                nc.tensor.matmul(
                    pt[:], inwT[k][:, m * P:(m + 1) * P],
                    xT[k][:, n * 512:(n + 1) * 512],
                    start=(k == 0), stop=(k == 3),
                )
            zst = sp.tile([P, 512], BF16, tag="zst", name="zst", bufs=2)
            nc.scalar.copy(zst[:], pt[:])
            nc.sync.dma_start(
                p["zscr"][(m - NC_D) * P:(m - NC_D + 1) * P, n * 512:(n + 1) * 512],
                zst[:],
            )
            yield


def _g23_units(tc, pools, p, d, st, m_lo, m_hi):
    """GEMM2 (when m_lo==0) + GEMM3/softplus for m in [m_lo, m_hi)."""
    nc = tc.nc
    psp = pools["psum"]
    ucT, dt_bf, bc_bf = st["ucT"], st["dt_bf"], st["bc_bf"]
    if m_lo == 0:
        for n in range(NN):
            pt = psp.tile([64, 512], F32, tag="g2", name="g2", bufs=1)
            for c in range(NC_D):
                nc.tensor.matmul(
                    pt[:], st["xpb"][:, c * 64:(c + 1) * 64],
                    ucT[c][:, n * 512:(n + 1) * 512],
                    start=(c == 0), stop=(c == NC_D - 1),
                )
            nc.scalar.copy(dt_bf[:, n * 512:(n + 1) * 512], pt[0:DT_RANK, :])
            nc.scalar.copy(bc_bf[:, n * 512:(n + 1) * 512], pt[DT_RANK:64, :])
            nc.sync.dma_start(p["bcscr"][:, n * 512:(n + 1) * 512],
                              bc_bf[:, n * 512:(n + 1) * 512])
            yield
    delta = st["delta"]
    for m in range(m_lo, m_hi):
        for n in range(NN):
            pt = psp.tile([P, 512], F32, tag="g3", name="g3", bufs=2)
            nc.tensor.matmul(
                pt[:], st["dtwT"][:, m * P:(m + 1) * P], dt_bf[:, n * 512:(n + 1) * 512],
                start=True, stop=True,
            )
            et = pools["sp"].tile([P, 512], F32, tag="sp_e", name="sp_e", bufs=2)
            nc.scalar.activation(et[:], pt[:], AF.Exp, bias=st["smf"][:, m * 20 + 18:m * 20 + 19])
            nc.scalar.activation(delta[m][:, n * 512:(n + 1) * 512], et[:], AF.Ln, bias=1.0)
        yield


def _wy_init(tc, pools, p, d, st):
    nc = tc.nc
    ucT, delta = st["ucT"], st["delta"]
    st["y"] = [pools["big"].tile([P, SEQ], BF16, tag="y", name=f"y{d}{c}", bufs=10) for c in range(NC_D)]
    for c in range(NC_D):
        nc.vector.tensor_mul(st["w"][c][:], delta[c][:], ucT[c][:])
        nc.vector.tensor_scalar(st["y"][c][:], ucT[c][:], st["smf"][:, c * 20 + 19:c * 20 + 20], None, op0=OP.mult)


def _scan_iter(tc, pools, st, p, s):
    """One state-index iteration of the selective scan."""
    nc = tc.nc
    bcp, ab = pools["bc"], pools["ab"]
    delta, w_bf, y_sb = st["delta"], st["w"], st["y"]
    smf = st["smf"]

    Bbc = bcp.tile([P, SEQ], BF16, tag="Bbc", name="Bbc", bufs=2)
    Cbc = bcp.tile([P, SEQ], BF16, tag="Cbc", name="Cbc", bufs=2)
    for src_row, dst in ((s, Bbc), (D_STATE + s, Cbc)):
        nc.sync.dma_start(
            dst[:], p["bcscr"][src_row:src_row + 1, :].broadcast_to((P, SEQ))
        )
    for c in range(NC_D):
        a_t = ab.tile([P, SEQ], BF16, tag="a", name="a", bufs=2)
        nc.scalar.activation(a_t[:], delta[c][:], AF.Exp, scale=smf[:, c * 20 + s:c * 20 + s + 1])
        b_t = ab.tile([P, SEQ], BF16, tag="b", name="b", bufs=2)
        nc.vector.tensor_mul(b_t[:], w_bf[c][:], Bbc[:])
        h_t = ab.tile([P, SEQ], BF16, tag="h", name="h", bufs=2)
        nc.vector.tensor_tensor_scan(
            h_t[:], a_t[:], b_t[:], 0.0, op0=OP.mult, op1=OP.add
        )
        pr = ab.tile([P, SEQ], BF16, tag="pr", name="pr", bufs=2)
        nc.gpsimd.tensor_tensor(pr[:], h_t[:], Cbc[:], op=OP.mult)
        nc.gpsimd.tensor_tensor(y_sb[c][:], y_sb[c][:], pr[:], op=OP.add)


def _gate_prep(tc, pools, st, p, d):
    """Load outwT and silu'd z (the zin tiles double as the g tiles)."""
    nc = tc.nc
    trans = pools["trans"]
    st["outwT"] = [trans.tile([P, D_MODEL], BF16, tag="outwT", name=f"outwT{d}{c}", bufs=8) for c in range(NC_D)]
    st["zin"] = []
    for c in range(NC_D):
        nc.sync.dma_start(st["outwT"][c][:], p["outwT"][c * P:(c + 1) * P, :])
        zin = trans.tile([P, SEQ], BF16, tag="zin", name=f"zin{d}{c}", bufs=8)
        nc.sync.dma_start(zin[:], p["zscr"][c * P:(c + 1) * P, :])
        st["zin"].append(zin)


def _gate_finish(tc, pools, st, p, d):
    nc = tc.nc
    psp = pools["psum"]
    y_sb, g, outwT = st["y"], st["zin"], st["outwT"]
    for c in range(NC_D):
        # y*silu(z) = y*z*(1+tanh(z/2))*0.5 -- the 0.5 lives in outwT
        th = pools["sp"].tile([P, SEQ], BF16, tag="th2", name="th2", bufs=1)
        nc.scalar.activation(th[:], g[c][:], AF.Tanh, scale=0.5)
        nc.vector.tensor_scalar(th[:], th[:], 1.0, None, op0=OP.add)
        nc.vector.tensor_mul(g[c][:], th[:], g[c][:])
        nc.vector.tensor_mul(g[c][:], y_sb[c][:], g[c][:])
    for m in range(SEQ // P):
        pt = psp.tile([P, D_MODEL], F32, tag="g4", name="g4", bufs=1)
        for c in range(NC_D):
            nc.tensor.matmul(
                pt[:], g[c][:, m * P:(m + 1) * P], outwT[c][:],
                start=(c == 0), stop=(c == NC_D - 1),
            )
        ot = pools["sp"].tile([P, D_MODEL], F32, tag="ot", name="ot", bufs=2)
        nc.scalar.copy(ot[:], pt[:])
        nc.sync.dma_start(p["out"][m * P:(m + 1) * P, :], ot[:])


def _split_excess_waits(nc):
    """walrus accepts at most one sync-wait per instruction (two for
    EventSemaphore); hoist the excess onto injected same-engine NoOps."""
    for f in nc.m.functions:
        for bb in f.blocks:
            new_insts = []
            for inst in bb.instructions:
                si = inst.sync_info
                cap = 2 if isinstance(inst, mybir.InstEventSemaphore) else 1
                if si is not None and len(si.on_wait) > cap:
                    waits = list(si.on_wait)
                    for i, wv in enumerate(waits[:-cap]):
                        nop = mybir.InstNoOp(name=f"{inst.name}-wsplit{i}", ins=[], outs=[])
                        nop.engine = inst.engine
                        nop.sync_info = bass_rust.SyncInfo(on_wait=[wv], on_update=[])
                        new_insts.append(nop)
                    inst.sync_info = bass_rust.SyncInfo(
                        on_wait=waits[-cap:], on_update=list(si.on_update)
                    )
                new_insts.append(inst)
            try:
                bb.instructions = new_insts
            except Exception:
                bb.instructions.clear()
                bb.instructions.extend(new_insts)


def build_bass():
    nc = bass.Bass()
    params = {d: _dir_params(nc, d) for d in ("f", "b")}
    with tile.TileContext(nc) as tc:
        with tc.tile_pool(name="cst", bufs=1) as cst, \
             tc.tile_pool(name="trans", bufs=2) as trans, \
             tc.tile_pool(name="big", bufs=10) as big, \
             tc.tile_pool(name="sp", bufs=2) as sp, \
             tc.tile_pool(name="bc", bufs=2) as bc, \
             tc.tile_pool(name="ab", bufs=2) as ab, \
             tc.tile_pool(name="psum", bufs=2, space="PSUM") as psum:
            pools = {"cst": cst, "trans": trans, "big": big, "sp": sp,
                     "bc": bc, "ab": ab, "psum": psum}
            # dir f prelude (u+conv; z-half deferred into the scan pump)
            st_f = _load_weights(tc, pools, params["f"], "f")
            for _ in _gemm1_conv_units(tc, pools, params["f"], "f", st_f,
                                       overlap=False, defer_z=True):
                pass
            for _ in _g23_units(tc, pools, params["f"], "f", st_f, 0, NC_D):
                pass
            _wy_init(tc, pools, params["f"], "f", st_f)

            # dir f scan, with f's z and dir b's GEMM1(u)+conv+z pumped in
            st_b = _load_weights(tc, pools, params["b"], "b")
            import itertools
            gen_b = itertools.chain(
                _z_units(tc, pools, params["f"], "f", st_f),
                _gemm1_conv_units(tc, pools, params["b"], "b", st_b,
                                  overlap=True, defer_z=True),
                _z_units(tc, pools, params["b"], "b", st_b),
                _g23_units(tc, pools, params["b"], "b", st_b, 0, NC_D),
            )
            for s in range(D_STATE):
                _scan_iter(tc, pools, st_f, params["f"], s)
                for _ in range(5):
                    if next(gen_b, StopIteration) is StopIteration:
                        break
            for _ in gen_b:
                pass

            _gate_prep(tc, pools, st_f, params["f"], "f")
            _gate_finish(tc, pools, st_f, params["f"], "f")
            _wy_init(tc, pools, params["b"], "b", st_b)
            _gate_prep(tc, pools, st_b, params["b"], "b")
            for s in range(D_STATE):
                _scan_iter(tc, pools, st_b, params["b"], s)
            _gate_finish(tc, pools, st_b, params["b"], "b")
    _split_excess_waits(nc)
    return nc


def _prep_dir(w):
    bf = ml_dtypes.bfloat16
    in_w, conv_w, conv_b, xp_w, dt_w, dt_b, A_log, Dp, out_w = w
    cw = np.asarray(conv_w, np.float32)
    convdiag = np.zeros((D_CONV, P, NC_D, P), np.float32)
    for k in range(D_CONV):
        for c in range(NC_D):
            convdiag[k, :, c, :] = np.diag(cw[c * P:(c + 1) * P, k])
    A = -np.exp(np.asarray(A_log, np.float64)).astype(np.float64)
    smf = np.zeros((NC_D, P, 20), np.float32)
    for c in range(NC_D):
        sl = slice(c * P, (c + 1) * P)
        smf[c, :, 0:16] = A[sl]
        smf[c, :, 16] = np.asarray(conv_b, np.float32)[sl]
        smf[c, :, 17] = smf[c, :, 16] * 0.5
        smf[c, :, 18] = np.asarray(dt_b, np.float32)[sl]
        smf[c, :, 19] = np.asarray(Dp, np.float32)[sl]
    xpT = np.ascontiguousarray(np.asarray(xp_w, np.float32).T)  # [D_INNER, 64]
    xpb = xpT.reshape(NC_D, P, 64).transpose(1, 0, 2).reshape(P, NC_D * 64)
    return {
        "inwT": np.ascontiguousarray(in_w.T).astype(bf),
        "xpb": np.ascontiguousarray(xpb).astype(bf),
        "dtwT": np.ascontiguousarray(dt_w.T).astype(bf),
        "outwT": (np.ascontiguousarray(out_w.T) * 0.5).astype(bf),
        "convdiag": convdiag.reshape(D_CONV * P, D_INNER).astype(bf),
        "smf": np.ascontiguousarray(smf.transpose(1, 0, 2).reshape(P, NC_D * 20)),
    }


_CACHED = {}


def kernel(
    x,
    in_w_f, conv_w_f, conv_b_f, xp_w_f, dt_w_f, dt_b_f, A_log_f, D_f, out_w_f,
    in_w_b, conv_w_b, conv_b_b, xp_w_b, dt_w_b, dt_b_b, A_log_b, D_b, out_w_b,
):
    bf = ml_dtypes.bfloat16
    x = np.asarray(x, dtype=np.float32)

    if "nc" not in _CACHED:
        _CACHED["nc"] = build_bass()
    nc = _CACHED["nc"]

    wf = _prep_dir((in_w_f, conv_w_f, conv_b_f, xp_w_f, dt_w_f, dt_b_f,
                    A_log_f, D_f, out_w_f))
    wb = _prep_dir((in_w_b, conv_w_b, conv_b_b, xp_w_b, dt_w_b, dt_b_b,
                    A_log_b, D_b, out_w_b))

    in_maps = []
    for b in range(BATCH):
        m = {}
        for d, wd in (("f", wf), ("b", wb)):
            for k, v in wd.items():
                m[f"{k}_{d}"] = v
        m["xT_f"] = np.ascontiguousarray(x[b].T).astype(bf)
        m["xT_b"] = np.ascontiguousarray(x[b][::-1].T).astype(bf)
        in_maps.append(m)

    res = run_bass_kernel_spmd(nc, in_maps, core_ids=list(range(BATCH)))
    out = np.empty((BATCH, SEQ, D_MODEL), np.float32)
    for b in range(BATCH):
        rb = res.results[b]
        out[b] = rb["out_f"] + rb["out_b"][::-1]
    return out
